# revision 8
# baseline (speedup 1.0000x reference)
"""Trainium2 Bass kernel: grouped similarity-gating normalization.

Reference computation (per batch b, group g, cpg=64 channels, hw=784):
    means[c]  = mean_hw(x[c, :])
    s[hw]     = sum_c x[c, hw] * means[c]
    t         = (s - mean(s)) * rsqrt(var(s) + eps)
    gate      = sigmoid(t * weight[g] + bias[g])
    out[c,hw] = x[c, hw] * gate[hw]

Sharding: data-parallel over batch B=64 across 8 cores (8 batches/core).

Per-core layout: one SBUF tile [128, 4, 784] per batch holds channels
c = 4*p + j (p = partition, j = free chunk) -> contiguous 1.6MB DMAs, and
group(c) = c//64 = p//16, i.e. each group owns a 16-partition band.

  - channel means via one DVE reduce (innermost axis of [128,4,784])
  - s (contraction over channels) via PE: 4 accumulating matmuls with
    lhsT[p, g] = means[p,j] masked to group bands (constant 0/1 indicator
    m8 times means). An extra N=1 matmul column with rhs=means gives
    mu = mean(s) = sum_c means[c]^2 for free.
  - stats on ScalarE: Square+accum_out -> sum(s^2); gate computed as
    sigmoid(s*a + c) in one activation with per-partition scale/bias APs,
    where a = rstd*weight[g], c = bias[g] - mu*a.
  - gate broadcast to the 128 partitions via PE with the transposed
    indicator (mt), then one DVE multiply (j-broadcast AP), DMA out.
"""

import sys

if "/opt/trn_rl_repo" not in sys.path:
    sys.path.insert(0, "/opt/trn_rl_repo")

from contextlib import ExitStack

import numpy as np

import concourse.bacc as bacc
import concourse.bass as bass
import concourse.tile as tile
from concourse import mybir
from concourse.bass_utils import run_bass_kernel_spmd

B, C, H, W = 64, 512, 28, 28
G = 8
HW = H * W          # 784
NCORES = 8
BLOC = B // NCORES  # 8 batches per core
NP = 128            # SBUF partitions
NJ = C // NP        # 4 channel chunks per partition (c = NJ*p + j)
PBAND = NP // G     # 16 partitions per group
EPS = 1e-5
F32 = mybir.dt.float32
MMCHUNK = 512       # max fp32 moving free dim per matmul

_cache: dict = {}


def _emit(tc, nc, xs, m8, mt, wv, bv, ys):
    AF = mybir.ActivationFunctionType
    with ExitStack() as ctx:
        consts = ctx.enter_context(tc.tile_pool(name="consts", bufs=1))
        xpool = ctx.enter_context(tc.tile_pool(name="xpool", bufs=BLOC))
        mpool = ctx.enter_context(tc.tile_pool(name="mpool", bufs=3))
        vpool = ctx.enter_context(tc.tile_pool(name="vpool", bufs=4))
        gpool = ctx.enter_context(tc.tile_pool(name="gpool", bufs=3))
        spsum = ctx.enter_context(tc.tile_pool(name="spsum", bufs=2, space="PSUM"))
        gpsum = ctx.enter_context(tc.tile_pool(name="gpsum", bufs=2, space="PSUM"))

        m8_sb = consts.tile([NP, G], F32)
        nc.sync.dma_start(out=m8_sb[:], in_=m8[:])
        mt_sb = consts.tile([G, NP], F32)
        nc.sync.dma_start(out=mt_sb[:], in_=mt[:])
        wv_sb = consts.tile([G, 1], F32)
        nc.sync.dma_start(out=wv_sb[:], in_=wv[:])
        bv_sb = consts.tile([G, 1], F32)
        nc.sync.dma_start(out=bv_sb[:], in_=bv[:])
        eps_sb = consts.tile([G, 1], F32)
        nc.vector.memset(eps_sb[:], EPS)

        for b in range(BLOC):
            xt = xpool.tile([NP, NJ, HW], F32)
            nc.sync.dma_start(out=xt[:], in_=xs[b])

            # per-channel spatial means
            sums = mpool.tile([NP, NJ], F32, tag="sums")
            nc.vector.reduce_sum(out=sums[:], in_=xt[:], axis=mybir.AxisListType.X)
            means = mpool.tile([NP, NJ], F32, tag="means")
            nc.vector.tensor_scalar_mul(means[:], sums[:], 1.0 / HW)

            # lhsT[:, j, g] = means[p, j] if p in group-g band else 0
            # (on ScalarE: TensorScalarPtr on DVE has a 1-sync-wait ISA limit)
            lhsT = mpool.tile([NP, NJ, G], F32, tag="lhsT")
            for j in range(NJ):
                nc.scalar.mul(lhsT[:, j, :], m8_sb[:], means[:, j : j + 1])

            # s[g, hw] (cols 0:HW) and mu[g] (col HW) via accumulating matmuls
            ps = spsum.tile([G, HW + 1], F32)
            for c0 in range(0, HW, MMCHUNK):
                c1 = min(c0 + MMCHUNK, HW)
                for j in range(NJ):
                    st = dict(start=(j == 0), stop=(j == NJ - 1))
                    nc.tensor.matmul(ps[:, c0:c1], lhsT[:, j, :], xt[:, j, c0:c1], **st)
            for j in range(NJ):
                st = dict(start=(j == 0), stop=(j == NJ - 1))
                nc.tensor.matmul(ps[:, HW : HW + 1], lhsT[:, j, :], means[:, j : j + 1], **st)

            # stats: ssq = sum(s^2), mu, var, rstd, a = rstd*w, c = b - mu*a
            sq = gpool.tile([G, HW], F32, tag="sq")
            ssq = vpool.tile([G, 1], F32, tag="ssq")
            nc.scalar.activation(out=sq[:], in_=ps[:, 0:HW], func=AF.Square, accum_out=ssq[:])
            mu = vpool.tile([G, 1], F32, tag="mu")
            nc.vector.tensor_copy(mu[:], ps[:, HW : HW + 1])
            musq = vpool.tile([G, 1], F32, tag="musq")
            nc.vector.tensor_mul(musq[:], mu[:], mu[:])
            var = vpool.tile([G, 1], F32, tag="var")
            nc.vector.scalar_tensor_tensor(
                out=var[:], in0=ssq[:], scalar=1.0 / HW, in1=musq[:],
                op0=mybir.AluOpType.mult, op1=mybir.AluOpType.subtract,
            )
            std = vpool.tile([G, 1], F32, tag="std")
            nc.scalar.activation(out=std[:], in_=var[:], func=AF.Sqrt, bias=eps_sb[:])
            rstd = vpool.tile([G, 1], F32, tag="rstd")
            nc.vector.reciprocal(rstd[:], std[:])
            a_t = vpool.tile([G, 1], F32, tag="a_t")
            nc.vector.tensor_mul(a_t[:], rstd[:], wv_sb[:])
            mua = vpool.tile([G, 1], F32, tag="mua")
            nc.vector.tensor_mul(mua[:], mu[:], a_t[:])
            c_t = vpool.tile([G, 1], F32, tag="c_t")
            nc.vector.tensor_sub(c_t[:], bv_sb[:], mua[:])

            # gate[g, hw] = sigmoid(s*a + c) straight from PSUM
            gate = gpool.tile([G, HW], F32, tag="gate")
            nc.scalar.activation(
                out=gate[:], in_=ps[:, 0:HW], func=AF.Sigmoid, bias=c_t[:], scale=a_t[:]
            )

            # broadcast gate rows to 16-partition bands: bg[p, hw] = gate[p//16, hw]
            bg = gpsum.tile([NP, HW], F32)
            for c0 in range(0, HW, MMCHUNK):
                c1 = min(c0 + MMCHUNK, HW)
                nc.tensor.matmul(bg[:, c0:c1], mt_sb[:], gate[:, c0:c1])

            # out = x * gate (bg broadcast over j), in place, then store
            bgb = bg[:, :].unsqueeze(1).broadcast_to([NP, NJ, HW])
            nc.vector.tensor_mul(xt[:], xt[:], bgb)
            nc.sync.dma_start(out=ys[b], in_=xt[:])


def _build_nc():
    nc = bacc.Bacc("TRN2", debug=False)
    xs = nc.dram_tensor("xs", [BLOC, NP, NJ, HW], F32, kind="ExternalInput")
    m8 = nc.dram_tensor("m8", [NP, G], F32, kind="ExternalInput")
    mt = nc.dram_tensor("mt", [G, NP], F32, kind="ExternalInput")
    wv = nc.dram_tensor("wv", [G, 1], F32, kind="ExternalInput")
    bv = nc.dram_tensor("bv", [G, 1], F32, kind="ExternalInput")
    ys = nc.dram_tensor("ys", [BLOC, NP, NJ, HW], F32, kind="ExternalOutput")
    with tile.TileContext(nc) as tc:
        _emit(tc, nc, xs, m8, mt, wv, bv, ys)
    nc.compile()
    return nc


def get_nc():
    if "nc" not in _cache:
        _cache["nc"] = _build_nc()
    return _cache["nc"]


def make_in_maps(x, weight, bias):
    x = np.ascontiguousarray(np.asarray(x, dtype=np.float32))
    weight = np.asarray(weight, dtype=np.float32).reshape(G)
    bias = np.asarray(bias, dtype=np.float32).reshape(G)
    # [core, b, p, j, hw] with c = NJ*p + j
    xs = x.reshape(NCORES, BLOC, NP, NJ, HW)
    m8 = np.zeros((NP, G), dtype=np.float32)
    m8[np.arange(NP), np.arange(NP) // PBAND] = 1.0
    mt = np.ascontiguousarray(m8.T)
    wv = np.ascontiguousarray(weight[:, None])
    bv = np.ascontiguousarray(bias[:, None])
    return [
        {"xs": np.ascontiguousarray(xs[i]), "m8": m8, "mt": mt, "wv": wv, "bv": bv}
        for i in range(NCORES)
    ]


def run(x, weight, bias, trace=False, **spmd_kwargs):
    nc = get_nc()
    in_maps = make_in_maps(x, weight, bias)
    res = run_bass_kernel_spmd(
        nc, in_maps, core_ids=list(range(NCORES)), trace=trace, **spmd_kwargs
    )
    out = np.stack([res.results[i]["ys"] for i in range(NCORES)])
    return out.reshape(B, C, H, W), res


def kernel(x, weight, bias, groups=G, **_ignored):
    assert int(groups) == G
    out, _ = run(x, weight, bias, trace=False)
    return out
